# revision 6
# baseline (speedup 1.0000x reference)
"""Trainium2 Bass kernel for nn_MemoryAttentionNetwork.

Sharding: pure data-parallel over batch (B=8 -> 8 cores). Each core gets its
batch slice of lmem/smem/hiddens plus replicated weights, computes its slice
of (out_mem, out_lmem); host stacks results.

Algorithm restructure vs reference:
- The kv projection over the big context `ah` (6208 rows/batch) is computed
  ONCE (not per memory-write iteration): softmax-over-n followed by k^T v
  equals (sum_n e^k [v|1]) / S, so we accumulate C_ah = E^T [V|1] in PSUM
  while streaming smem/hiddens tiles, and per-iteration only add the 256
  lmem-row contribution.
- depth_emb (scalar per depth) is added on ScalarE during the stream; the
  out_mem LayerNorm is shift-invariant so it reads the same tiles.
- out_mem LN is fused into the same stream (single pass over smem/hiddens).
- Matmuls run in bf16 (inputs cast on-chip); accumulation fp32 in PSUM.
"""

import sys

sys.path.insert(0, "/opt/trn_rl_repo")

import numpy as np

import concourse.bass as bass
import concourse.mybir as mybir
import concourse.tile as tile
from concourse import bacc
from concourse.bass_utils import run_bass_kernel_spmd

F32 = mybir.dt.float32
BF16 = mybir.dt.bfloat16
AF = mybir.ActivationFunctionType
ALU = mybir.AluOpType

DIM = 1024
HEADS = 8
DEPTH = 4
MEM_LEN = 1024
LMEM_LEN = 256
NUM_MEM_KV = 64
ITERS = 2
B = 8
SEQ = 512
KC = DIM // 128  # contraction chunks
DH = 128  # head dim
EPS = 1e-5

GATE_NAMES = ["Ua", "Wa", "Uc", "Wc", "U"]


def _ln_tile(nc, pool, xe, out, g_bc=None, b_bc=None, n_part=128, eps_t=None):
    """LayerNorm over free dim (1024) of xe[128,1024] -> out (f32).
    If g_bc/b_bc given, apply affine. Returns nothing (writes out)."""
    p = n_part
    stats = pool.tile([128, 2, 6], F32, tag="ln_stats")
    for sg in range(2):
        nc.vector.bn_stats(out=stats[:p, sg, :], in_=xe[:p, sg * 512:(sg + 1) * 512])
    mv = pool.tile([128, 2], F32, tag="ln_mv")
    nc.vector.bn_aggr(out=mv[:p], in_=stats[:p])
    # rstd = 1/sqrt(var+eps)
    rstd = pool.tile([128, 1], F32, tag="ln_rstd")
    nc.scalar.activation(out=rstd[:p], in_=mv[:p, 1:2], func=AF.Sqrt, bias=eps_t[:p])
    nc.vector.reciprocal(out=rstd[:p], in_=rstd[:p])
    nc.vector.tensor_scalar(
        out=out[:p], in0=xe[:p], scalar1=mv[:p, 0:1], scalar2=rstd[:p],
        op0=ALU.subtract, op1=ALU.mult)
    if g_bc is not None:
        nc.vector.tensor_mul(out=out[:p], in0=out[:p], in1=g_bc[:p])
        nc.vector.tensor_add(out=out[:p], in0=out[:p], in1=b_bc[:p])


def _transpose8(nc, pool, x16, tag, n_part=128, bufs=2):
    """x16 [128,1024] bf16 -> xT [128, 8, 128] bf16 via 8 xbar DMA transposes."""
    xT = pool.tile([128, KC, n_part], BF16, tag=tag, bufs=bufs)
    for c in range(KC):
        nc.scalar.dma_start(out=xT[:, c, :], in_=x16[:n_part, c * 128:(c + 1) * 128],
                            transpose=True)
    return xT


def build_nc():
    nc = bacc.Bacc(None, target_bir_lowering=False)

    din = {}
    def inp(name, shape):
        din[name] = nc.declare_dram_parameter(name, list(shape), F32, isOutput=False)
        return din[name]

    lmem_d = inp("lmem", (LMEM_LEN, DIM))
    smem_d = inp("smem", (DEPTH, MEM_LEN, DIM))
    hid_d = inp("hiddens", (DEPTH, SEQ, DIM))
    wq_d = inp("w_q", (DIM, DIM))
    wkv_d = inp("w_kv", (DIM, 2 * DIM))
    wout_d = inp("w_out", (DIM, DIM))
    demb_d = inp("depth_emb", (DEPTH,))
    lpos_d = inp("lmem_pos_emb", (LMEM_LEN, DIM))
    mkv_d = inp("mem_kv", (NUM_MEM_KV, DIM))
    for gn in GATE_NAMES:
        inp(gn + "_w", (DIM, DIM))
        inp(gn + "_b", (DIM,))
    n1g_d = inp("norm1_g", (DIM,))
    n1b_d = inp("norm1_b", (DIM,))
    n2g_d = inp("norm2_g", (DIM,))
    n2b_d = inp("norm2_b", (DIM,))

    omem_d = nc.declare_dram_parameter("out_mem", [DEPTH, MEM_LEN, DIM], F32, isOutput=True)
    olmem_d = nc.declare_dram_parameter("out_lmem", [LMEM_LEN, DIM], F32, isOutput=True)

    # bf16 bounce buffers in DRAM for phase-2 weights
    # layout [8 (chunk), 128, 1024]: chunk c holds rows c*128..c*128+127 of the
    # (possibly transposed) matrix; partition dim = row % 128.
    wq_b_d = nc.dram_tensor("wq_bf", [KC, 128, DIM], BF16)
    wout_b_d = nc.dram_tensor("wout_bf", [KC, 128, DIM], BF16)
    gate_bf_d = {gn: nc.dram_tensor(f"wt_{gn}_bf", [KC, 128, DIM], BF16)
                 for gn in GATE_NAMES}

    with tile.TileContext(nc) as tc:
        import contextlib
        with contextlib.ExitStack() as ctx:
            singles = ctx.enter_context(tc.tile_pool(name="singles", bufs=1))

            # ---- persistent small tensors ----
            embb = singles.tile([128, DEPTH], F32)
            nc.sync.dma_start(out=embb, in_=demb_d[:].partition_broadcast(128))
            n1g = singles.tile([128, DIM], F32)
            nc.sync.dma_start(out=n1g, in_=n1g_d[None, :].broadcast_to([128, DIM]))
            n1b = singles.tile([128, DIM], F32)
            nc.sync.dma_start(out=n1b, in_=n1b_d[None, :].broadcast_to([128, DIM]))

            # w_kv resident bf16 [128, kc, 2048]
            wkv_b = singles.tile([128, KC, 2 * DIM], BF16)

            # C_ah accumulator (SBUF copy after stream) [128, 8, 132]
            C_ah = singles.tile([128, HEADS, 132], F32)
            eps_t = singles.tile([128, 1], F32)
            nc.vector.memset(eps_t, EPS)

            # ---- phase 0: weight prep ----
            with tc.tile_pool(name="wprep", bufs=3) as wp:
                for kc in range(KC):
                    st = wp.tile([128, 2 * DIM], F32, tag="wkv_st")
                    nc.sync.dma_start(out=st, in_=wkv_d[kc * 128:(kc + 1) * 128, :])
                    nc.vector.tensor_copy(out=wkv_b[:, kc, :], in_=st)
                # w_q / w_out: cast-only bounce (natural layout)
                for src_d, dst_d in ((wq_d, wq_b_d), (wout_d, wout_b_d)):
                    for kc in range(KC):
                        st = wp.tile([128, DIM], F32, tag="w_st")
                        nc.sync.dma_start(out=st, in_=src_d[kc * 128:(kc + 1) * 128, :])
                        c16 = wp.tile([128, DIM], BF16, tag="w_c16")
                        nc.vector.tensor_copy(out=c16, in_=st)
                        nc.sync.dma_start(out=dst_d[kc, :, :], in_=c16)
                # gate weights: transpose bounce. W [i, j] -> WT[j, i]
                for gn in GATE_NAMES:
                    wsrc = din[gn + "_w"]
                    wdst = gate_bf_d[gn]
                    for rc in range(KC):  # row tile of W (i block)
                        st = wp.tile([128, DIM], F32, tag="w_st")
                        nc.sync.dma_start(out=st, in_=wsrc[rc * 128:(rc + 1) * 128, :])
                        c16 = wp.tile([128, DIM], BF16, tag="w_c16")
                        nc.vector.tensor_copy(out=c16, in_=st)
                        # transpose each [128,128] block: block cc -> WT chunk cc, cols rc
                        tt = wp.tile([128, KC, 128], BF16, tag="w_tt")
                        for cc in range(KC):
                            nc.scalar.dma_start(out=tt[:, cc, :],
                                                in_=c16[:, cc * 128:(cc + 1) * 128],
                                                transpose=True)
                        nc.sync.dma_start(out=wdst[:, :, rc * 128:(rc + 1) * 128]
                                          .rearrange("c p n -> p c n"), in_=tt)

            # ---- phase 1: stream smem/hiddens; LN -> out_mem; kv proj -> C_ah ----
            with tc.tile_pool(name="s_pool", bufs=3) as sp, \
                 tc.tile_pool(name="s_pool2", bufs=2) as sp2, \
                 tc.tile_pool(name="ps_proj", bufs=4, space="PSUM") as pp, \
                 tc.tile_pool(name="ps_c", bufs=1, space="PSUM") as pc:

                C_ps = []
                for g in range(4):
                    C_ps.append(pc.tile([128, 2, 129], F32, tag=f"C{g}", name=f"Cps{g}"))

                # stream tile list: (src_ap, depth_idx, out_row or None, n_part)
                tiles = []
                for d in range(DEPTH):
                    for r in range(MEM_LEN // 128):  # smem rows
                        orow = None if r < 4 else (r - 4) * 128
                        tiles.append((smem_d[d, r * 128:(r + 1) * 128, :], d, orow, 128))
                    for r in range(SEQ // 128):  # hiddens rows
                        tiles.append((hid_d[d, r * 128:(r + 1) * 128, :], d,
                                      SEQ + r * 128, 128))
                tiles.append((mkv_d[:, :], None, None, NUM_MEM_KV))

                n_t = len(tiles)
                for ti, (src, d, orow, npart) in enumerate(tiles):
                    x = sp.tile([128, DIM], F32, tag="x")
                    nc.sync.dma_start(out=x[:npart], in_=src)
                    xe = sp2.tile([128, DIM], F32, tag="xe")
                    if d is not None:
                        nc.scalar.activation(out=xe[:npart], in_=x[:npart],
                                             func=AF.Identity, bias=embb[:, d:d + 1])
                    else:
                        nc.scalar.activation(out=xe[:npart], in_=x[:npart],
                                             func=AF.Identity, bias=0.0)
                    # LN/output path
                    if orow is not None:
                        on = sp2.tile([128, DIM], F32, tag="on")
                        _ln_tile(nc, sp2, xe, on, n1g, n1b, eps_t=eps_t)
                        nc.sync.dma_start(out=omem_d[d, orow:orow + 128, :], in_=on)
                    # kv path
                    x16 = sp2.tile([128, DIM], BF16, tag="x16")
                    nc.vector.tensor_copy(out=x16[:npart], in_=xe[:npart])
                    xT = _transpose8(nc, sp2, x16, "xT", n_part=npart)
                    E16 = sp2.tile([128, DIM], BF16, tag="E16")
                    Vp = sp2.tile([128, HEADS, 132], BF16, tag="Vp")
                    nc.vector.memset(Vp[:npart, :, 128:129], 1.0)
                    for cg in range(4):
                        ps = pp.tile([128, 512], F32, tag="proj")
                        for kc in range(KC):
                            nc.tensor.matmul(ps[:npart], xT[:, kc, :npart],
                                             wkv_b[:, kc, cg * 512:(cg + 1) * 512],
                                             start=(kc == 0), stop=(kc == KC - 1))
                        if cg < 2:  # k half -> exp
                            nc.scalar.activation(out=E16[:npart, cg * 512:(cg + 1) * 512],
                                                 in_=ps[:npart], func=AF.Exp)
                        else:  # v half -> packed copy (4 heads per 512)
                            h0 = (cg - 2) * 4
                            nc.scalar.activation(
                                out=Vp[:npart, h0:h0 + 4, 0:128],
                                in_=ps[:npart], func=AF.Copy)
                    for h in range(HEADS):
                        nc.tensor.matmul(
                            C_ps[h // 2][:, h % 2, :],
                            E16[:npart, h * 128:(h + 1) * 128],
                            Vp[:npart, h, 0:129],
                            start=(ti == 0), stop=(ti == n_t - 1))

                for g in range(4):
                    nc.vector.tensor_copy(out=C_ah[:, 2 * g:2 * g + 2, 0:129],
                                          in_=C_ps[g][:, :, :])

            # ---- phase 2: lmem attention + gate, 2 iterations ----
            with tc.tile_pool(name="l1", bufs=1) as l1, \
                 tc.tile_pool(name="l2", bufs=1) as l2, \
                 tc.tile_pool(name="ps_mm", bufs=4, space="PSUM") as pm, \
                 tc.tile_pool(name="ps_cnx", bufs=1, space="PSUM") as pcx:

                h_t = l1.tile([128, 2, DIM], F32)  # current lmem (2 rowblocks)
                for rb in range(2):
                    hx = l2.tile([128, DIM], F32, tag="hx")
                    nc.sync.dma_start(out=hx, in_=lmem_d[rb * 128:(rb + 1) * 128, :])
                    pos = l2.tile([128, DIM], F32, tag="pos")
                    nc.sync.dma_start(out=pos, in_=lpos_d[rb * 128:(rb + 1) * 128, :])
                    nc.vector.tensor_add(out=h_t[:, rb, :], in0=hx, in1=pos)

                # gate biases (pa = Ua_b + Wa_b etc), as bf16 rows [1, 1024]
                brow = {}
                for nm, pair in (("pa", ("Ua_b", "Wa_b")), ("pc", ("Uc_b", "Wc_b")),
                                 ("pu", ("U_b", None))):
                    b0 = l2.tile([1, DIM], F32, tag="b0")
                    nc.sync.dma_start(out=b0, in_=din[pair[0]][None, :])
                    if pair[1] is not None:
                        b1 = l2.tile([1, DIM], F32, tag="b1")
                        nc.sync.dma_start(out=b1, in_=din[pair[1]][None, :])
                        nc.vector.tensor_add(out=b0, in0=b0, in1=b1)
                    bb = l1.tile([1, DIM], BF16, tag=f"bias_{nm}")
                    nc.vector.tensor_copy(out=bb, in_=b0)
                    brow[nm] = bb
                ones16 = l1.tile([1, 128], BF16)
                nc.vector.memset(ones16, 1.0)

                n2g = l1.tile([128, DIM], F32)
                nc.sync.dma_start(out=n2g, in_=n2g_d[None, :].broadcast_to([128, DIM]))
                n2b = l1.tile([128, DIM], F32)
                nc.sync.dma_start(out=n2b, in_=n2b_d[None, :].broadcast_to([128, DIM]))

                def load_wbuf(src_d):
                    wb = l2.tile([128, KC, DIM], BF16, tag="wbuf", bufs=2)
                    nc.sync.dma_start(out=wb, in_=src_d.rearrange("c p n -> p c n"))
                    return wb

                for it in range(ITERS):
                    nxT = []  # per rowblock [128, kc, 128]
                    for rb in range(2):
                        nx = l2.tile([128, DIM], F32, tag="nx")
                        _ln_tile(nc, l2, h_t[:, rb, :], nx, eps_t=eps_t)
                        nx16 = l2.tile([128, DIM], BF16, tag="nx16")
                        nc.vector.tensor_copy(out=nx16, in_=nx)
                        nxT.append(_transpose8(nc, l2, nx16, f"nxT{rb}"))

                    # ---- q proj + softmax(dh) -> qT16 [128, h, rb, 128]
                    wq_b = load_wbuf(wq_b_d)
                    qT = l2.tile([128, HEADS, 2, 128], BF16, tag="qT")
                    for rb in range(2):
                        Eq = l2.tile([128, DIM], F32, tag="Eq")
                        for cg in range(2):
                            ps = pm.tile([128, 512], F32, tag="mm")
                            for kc in range(KC):
                                nc.tensor.matmul(ps, nxT[rb][:, kc, :],
                                                 wq_b[:, kc, cg * 512:(cg + 1) * 512],
                                                 start=(kc == 0), stop=(kc == KC - 1))
                            nc.scalar.activation(out=Eq[:, cg * 512:(cg + 1) * 512],
                                                 in_=ps, func=AF.Exp)
                        qs = l2.tile([128, HEADS], F32, tag="qs")
                        nc.vector.tensor_reduce(
                            out=qs, in_=Eq.rearrange("p (h d) -> p h d", h=HEADS),
                            axis=mybir.AxisListType.X, op=ALU.add)
                        nc.vector.reciprocal(out=qs, in_=qs)
                        q16 = l2.tile([128, DIM], BF16, tag="q16")
                        for h in range(HEADS):
                            nc.vector.tensor_scalar_mul(
                                out=q16[:, h * 128:(h + 1) * 128],
                                in0=Eq[:, h * 128:(h + 1) * 128], scalar1=qs[:, h:h + 1])
                        for h in range(HEADS):
                            nc.scalar.dma_start(out=qT[:, h, rb, :],
                                                in_=q16[:, h * 128:(h + 1) * 128],
                                                transpose=True)

                    # ---- kv proj for nx rows; C_nx accumulation
                    C_nx = []
                    for g in range(4):
                        C_nx.append(pcx.tile([128, 2, 129], F32, tag=f"Cnx{g}", name=f"Cnx{g}_{it}"))
                    for rb in range(2):
                        E16 = l2.tile([128, DIM], BF16, tag="E16l")
                        Vp = l2.tile([128, HEADS, 132], BF16, tag="Vpl")
                        nc.vector.memset(Vp[:, :, 128:129], 1.0)
                        for cg in range(4):
                            ps = pm.tile([128, 512], F32, tag="mm")
                            for kc in range(KC):
                                nc.tensor.matmul(ps, nxT[rb][:, kc, :],
                                                 wkv_b[:, kc, cg * 512:(cg + 1) * 512],
                                                 start=(kc == 0), stop=(kc == KC - 1))
                            if cg < 2:
                                nc.scalar.activation(out=E16[:, cg * 512:(cg + 1) * 512],
                                                     in_=ps, func=AF.Exp)
                            else:
                                h0 = (cg - 2) * 4
                                nc.scalar.activation(out=Vp[:, h0:h0 + 4, 0:128],
                                                     in_=ps, func=AF.Copy)
                        for h in range(HEADS):
                            nc.tensor.matmul(C_nx[h // 2][:, h % 2, :],
                                             E16[:, h * 128:(h + 1) * 128],
                                             Vp[:, h, 0:129],
                                             start=(rb == 0), stop=(rb == 1))

                    # ---- ctx = (C_ah + C_nx) / S
                    ctx_f = l2.tile([128, HEADS, 129], F32, tag="ctxf")
                    for g in range(4):
                        nc.vector.tensor_add(out=ctx_f[:, 2 * g:2 * g + 2, :],
                                             in0=C_nx[g][:, :, :],
                                             in1=C_ah[:, 2 * g:2 * g + 2, 0:129])
                    sr = l2.tile([128, HEADS], F32, tag="sr")
                    nc.vector.tensor_copy(out=sr, in_=ctx_f[:, :, 128])
                    nc.vector.reciprocal(out=sr, in_=sr)
                    ctx16 = l2.tile([128, HEADS, 128], BF16, tag="ctx16")
                    for h in range(HEADS):
                        nc.vector.tensor_scalar_mul(out=ctx16[:, h, :],
                                                    in0=ctx_f[:, h, 0:128],
                                                    scalar1=sr[:, h:h + 1])

                    # ---- out^T = ctx_h^T @ q_h^T  [e,128] x [d,256] -> [e? ]
                    attnT = l2.tile([128, HEADS, 256], BF16, tag="attnT")
                    for h in range(HEADS):
                        ps = pm.tile([128, 256], F32, tag="mm")
                        nc.tensor.matmul(ps, ctx16[:, h, :], qT[:, h, :, :])
                        nc.scalar.activation(out=attnT[:, h, :], in_=ps, func=AF.Copy)

                    # ---- attn_out^T = w_out^T-chunks @ attnT  -> xg [128, ec, 256]
                    wout_b = load_wbuf(wout_b_d)
                    xg = l2.tile([128, KC, 256], BF16, tag="xg")
                    for ec in range(KC):
                        ps = pm.tile([128, 256], F32, tag="mm")
                        for dc in range(KC):
                            nc.tensor.matmul(ps, wout_b[:, dc, ec * 128:(ec + 1) * 128],
                                             attnT[:, dc, :],
                                             start=(dc == 0), stop=(dc == KC - 1))
                        nc.scalar.activation(out=xg[:, ec, :], in_=ps, func=AF.Copy)

                    # ---- h^T for gate
                    hT = l2.tile([128, KC, 256], BF16, tag="hT")
                    for rb in range(2):
                        h16 = l2.tile([128, DIM], BF16, tag="h16")
                        nc.vector.tensor_copy(out=h16, in_=h_t[:, rb, :])
                        for jc in range(KC):
                            nc.scalar.dma_start(
                                out=hT[:, jc, rb * 128:(rb + 1) * 128],
                                in_=h16[:, jc * 128:(jc + 1) * 128], transpose=True)

                    # ---- gate: stream weights one at a time into SBUF f32 accs
                    pa = l2.tile([128, 2, DIM], F32, tag="pa")
                    pc_a = l2.tile([128, 2, DIM], F32, tag="pc_a")
                    pu = l2.tile([128, 2, DIM], F32, tag="pu")
                    accs = {"Ua": (pa, "pa", True), "Wa": (pa, None, False),
                            "Uc": (pc_a, "pc", True), "Wc": (pc_a, None, False),
                            "U": (pu, "pu", True)}
                    for gn in GATE_NAMES:
                        acc, bias_nm, first = accs[gn]
                        wb = load_wbuf(gate_bf_d[gn])
                        lhs = xg if gn[0] == "U" else hT
                        for rb in range(2):
                            for cg in range(2):
                                sl = slice(cg * 512, (cg + 1) * 512)
                                ps = pm.tile([128, 512], F32, tag="mm")
                                if first:
                                    nc.tensor.matmul(
                                        ps, ones16, brow[bias_nm][:, sl],
                                        start=True, stop=False)
                                for jc in range(KC):
                                    nc.tensor.matmul(
                                        ps, lhs[:, jc, rb * 128:(rb + 1) * 128],
                                        wb[:, jc, sl],
                                        start=(not first and jc == 0),
                                        stop=(jc == KC - 1))
                                if first:
                                    nc.vector.tensor_copy(out=acc[:, rb, sl], in_=ps)
                                else:
                                    nc.vector.tensor_add(out=acc[:, rb, sl],
                                                         in0=acc[:, rb, sl], in1=ps)
                    for rb in range(2):
                        nc.scalar.activation(out=pa[:, rb, :], in_=pa[:, rb, :], func=AF.Tanh)
                        nc.scalar.activation(out=pc_a[:, rb, :], in_=pc_a[:, rb, :],
                                             func=AF.Sigmoid)
                    for rb in range(2):
                        for cg in range(2):
                            sl = slice(cg * 512, (cg + 1) * 512)
                            # u_arg = pu + (1 + a)*h ; t = tanh(u_arg)
                            w1 = l2.tile([128, 512], F32, tag="w1", bufs=2)
                            nc.vector.scalar_tensor_tensor(
                                out=w1, in0=pa[:, rb, sl], scalar=1.0,
                                in1=h_t[:, rb, sl], op0=ALU.add, op1=ALU.mult)
                            nc.vector.tensor_add(out=w1, in0=w1, in1=pu[:, rb, sl])
                            tt = l2.tile([128, 512], F32, tag="tt", bufs=2)
                            nc.scalar.activation(out=tt, in_=w1, func=AF.Tanh)
                            # h_new = c*(h - t) + t
                            dd = l2.tile([128, 512], F32, tag="dd", bufs=2)
                            nc.vector.tensor_sub(out=dd, in0=h_t[:, rb, sl], in1=tt)
                            nc.vector.tensor_mul(out=dd, in0=pc_a[:, rb, sl], in1=dd)
                            nc.vector.tensor_add(out=h_t[:, rb, sl], in0=dd, in1=tt)

                # ---- out_lmem = LN(h)*g2 + b2 ----
                for rb in range(2):
                    ol = l2.tile([128, DIM], F32, tag="ol")
                    _ln_tile(nc, l2, h_t[:, rb, :], ol, n2g, n2b, eps_t=eps_t)
                    nc.sync.dma_start(out=olmem_d[rb * 128:(rb + 1) * 128, :], in_=ol)

    nc.finalize()
    return nc


_NC_CACHE = None


def kernel(**inputs):
    global _NC_CACHE
    if _NC_CACHE is None:
        _NC_CACHE = build_nc()
    nc = _NC_CACHE

    f = lambda a: np.ascontiguousarray(np.asarray(a, dtype=np.float32))
    in_maps = []
    for b in range(B):
        m = {
            "lmem": f(inputs["lmem"][b]),
            "smem": f(inputs["smem"][:, b]),
            "hiddens": f(inputs["hiddens"][:, b]),
            "w_q": f(inputs["w_q"]),
            "w_kv": f(inputs["w_kv"]),
            "w_out": f(inputs["w_out"]),
            "depth_emb": f(inputs["depth_emb"]).reshape(DEPTH),
            "lmem_pos_emb": f(inputs["lmem_pos_emb"]).reshape(LMEM_LEN, DIM),
            "mem_kv": f(inputs["mem_kv"]).reshape(NUM_MEM_KV, DIM),
            "norm1_g": f(inputs["norm1_g"]), "norm1_b": f(inputs["norm1_b"]),
            "norm2_g": f(inputs["norm2_g"]), "norm2_b": f(inputs["norm2_b"]),
        }
        for gn in GATE_NAMES:
            m[gn + "_w"] = f(inputs[gn + "_w"])
            m[gn + "_b"] = f(inputs[gn + "_b"])
        in_maps.append(m)

    res = run_bass_kernel_spmd(nc, in_maps, list(range(B))).results
    out_mem = np.stack([res[b]["out_mem"] for b in range(B)], axis=1)
    out_lmem = np.stack([res[b]["out_lmem"] for b in range(B)], axis=0)
    return (out_mem, out_lmem)
